# revision 3
# baseline (speedup 1.0000x reference)
"""Trainium2 Bass kernel: per-(head,batch) euclidean compatibility matrix,
globally min/max-rescaled to [-9, 9].

reference (jax):
    q_sq = sum(Q*Q, -1)[..., :, None]
    k_sq = sum(K*K, -1)[..., None, :]
    cross = einsum("hbqd,hbgd->hbqg", Q, K)
    compat = sqrt(q_sq + k_sq - 2*cross)
    out = A_LO + (compat - min) * (A_HI - A_LO) / (max - min)   # min/max per (h,b)

Sharding: head h -> NeuronCore h (8 heads, 8 cores), fully independent.

Per-core program (B=4 slices of [N=2048, D=16]):
  phase A (all slices up front): load Q/K in natural layout [128, 16, 18]
    (fp32), compute row sum-of-squares; K gets (k_hi, k_lo) fp16-split
    sum-of-squares columns, Q gets const -0.5 columns and a separate
    per-partition q_sq tile (fp32, fed to ACT as bias).  PE-transposes
    build fp16 UT = [Q^T; -.5; -.5], VT = [K^T; k_hi; k_lo], so
      psum = UT[:,q]^T @ VT[:,g] = QK - 0.5*k_sq
      d2   = -2*psum + q_sq      (exact-ish; ACT pre-affine, scale=-2)
  phase B: per supertile (4 q-tiles, [128, 8192] fp16): 16 fp16 matmuls
    -> PSUM, ACT sqrt(scale*x+bias) PSUM->fp16 SBUF, 2 fused DVE
    tensor_scalar+accum reduces (min via negate / max partials).
  phase C: finalize min/max across supertiles + partitions (gpsimd
    all-reduce), c1 = 18/(max-min), c0 = -9 + (-min)*c1.
  phase D: per supertile: DVE madd (fp16, 4x mode), DMA to DRAM fp16.

Output is fp16 on device (halves HBM write traffic); host upcasts to
fp32.  rel-err budget 2e-2 >> fp16 quantization (~1e-3).
"""

import numpy as np

H, B, N, D = 8, 4, 2048, 16
A_LO, A_HI = -9.0, 9.0
P = 128
R = D + 2            # matmul contraction rows: 16 data + k_hi + k_lo
NT = N // P          # 16 q-tiles per slice
STW = 4              # q-tiles per supertile
ST = NT // STW       # 4 supertiles per slice

# ---- tuning knobs ----
SQ_BUFS = 8          # SBUF supertile bufs of [128, 8192] fp16
ACT_COPIES = 4       # of the 8 phase-A PSUM->SBUF copies routed to ACT (rest DVE)
AFF_ACT = 0          # of ST*B affines routed to ACT (rest DVE)

_CACHE = {}


def build_program():
    import concourse.bacc as bacc
    import concourse.bass as bass
    import concourse.mybir as mybir
    from concourse import tile, masks
    from concourse import bass_isa

    f32 = mybir.dt.float32
    f16 = mybir.dt.float16
    Alu = mybir.AluOpType
    AF = mybir.ActivationFunctionType
    AX = mybir.AxisListType

    nc = bacc.Bacc()
    Qd = nc.declare_dram_parameter("Q", [B, N, D], f32, isOutput=False)
    Kd = nc.declare_dram_parameter("K", [B, N, D], f32, isOutput=False)
    Od = nc.declare_dram_parameter("out", [B, N, N], f16, isOutput=True)

    with tile.TileContext(nc) as tc:
        with (
            tc.tile_pool(name="const", bufs=1) as constp,
            tc.tile_pool(name="ld", bufs=2) as ldp,
            tc.tile_pool(name="sqt", bufs=2) as sqtp,
            tc.tile_pool(name="qsq", bufs=4) as qsqp,
            tc.tile_pool(name="ksq", bufs=2) as ksqp,
            tc.tile_pool(name="uv", bufs=8) as uvp,
            tc.tile_pool(name="sq", bufs=SQ_BUFS) as sqp,
            tc.tile_pool(name="dmy", bufs=2) as dmyp,
            tc.tile_pool(name="small", bufs=2) as smallp,
            tc.tile_pool(name="ps", bufs=2, space=bass.MemorySpace.PSUM) as psp,
        ):
            ident = constp.tile([P, P], f32)
            masks.make_identity(nc, ident[:])

            # ---------------- phase A: build UT / VT / qsq for all slices ----
            UTs, VTs, qsqs = [], [], []
            copy_idx = 0
            for b in range(B):
                for (src, is_k) in ((Qd, False), (Kd, True)):
                    ld = ldp.tile([P, NT, R], f32, tag="ld")
                    if not is_k:
                        # const -0.5 in cols D, D+1 (survives the data DMA)
                        nc.gpsimd.memset(ld[:], -0.5)
                    nc.sync.dma_start(
                        ld[:, :, 0:D], src[b].rearrange("(t p) d -> p t d", p=P)
                    )
                    sqt = sqtp.tile([P, NT, D], f32, tag="sqt")
                    nc.vector.tensor_tensor(
                        sqt[:], ld[:, :, 0:D], ld[:, :, 0:D], Alu.mult
                    )
                    if is_k:
                        ksq = ksqp.tile([P, NT], f32, tag="ksq")
                        nc.vector.tensor_reduce(ksq[:], sqt[:], AX.X, Alu.add)
                        # fp16 hi/lo split of k_sq -> ld cols D, D+1
                        khil = ksqp.tile([P, NT], f16, tag="khil")
                        nc.vector.tensor_copy(khil[:], ksq[:])
                        nc.vector.tensor_copy(ld[:, :, D], khil[:])
                        nc.vector.tensor_tensor(
                            ld[:, :, D + 1], ksq[:], ld[:, :, D], Alu.subtract
                        )
                    else:
                        qsq = qsqp.tile([P, NT], f32, tag="qsq")
                        nc.vector.tensor_reduce(qsq[:], sqt[:], AX.X, Alu.add)
                        qsqs.append(qsq)
                    # transpose [128, 18] chunks -> [18, 2048] fp16 in SBUF
                    tp = psp.tile([R, N], f32, tag="ps")
                    for t in range(NT):
                        nc.tensor.transpose(
                            tp[:, t * P : (t + 1) * P], ld[:, t, :], ident[:]
                        )
                    TT = uvp.tile([R, N], f16, tag="uv")
                    if copy_idx < ACT_COPIES:
                        nc.scalar.copy(TT[:], tp[:])
                    else:
                        nc.vector.tensor_copy(TT[:], tp[:])
                    copy_idx += 1
                    (VTs if is_k else UTs).append(TT)

            # ---------------- phases B-D per slice ----------------
            for b in range(B):
                UT, VT, qsq = UTs[b], VTs[b], qsqs[b]
                minp = smallp.tile([P, ST], f32, tag="minp")
                maxp = smallp.tile([P, ST], f32, tag="maxp")
                sts = []
                for s in range(ST):
                    st = sqp.tile([P, STW * N], f16, tag="sq")
                    for t in range(STW):
                        i = s * STW + t
                        d2 = psp.tile([P, N], f32, tag="ps")
                        lhs = UT[:, i * P : (i + 1) * P]
                        for j in range(4):
                            nc.tensor.matmul(
                                d2[:, j * 512 : (j + 1) * 512],
                                lhs,
                                VT[:, j * 512 : (j + 1) * 512],
                                start=True,
                                stop=True,
                            )
                        # sq = sqrt(-2*psum + q_sq) = euclidean distance
                        nc.scalar.activation(
                            st[:, t * N : (t + 1) * N],
                            d2[:],
                            AF.Sqrt,
                            bias=qsq[:, i : i + 1],
                            scale=-2.0,
                        )
                    # fused reduces: minp holds NEGATED minima (max of -sq)
                    dm0 = dmyp.tile([P, 1], f16, tag="dmy")
                    nc.vector.tensor_scalar(
                        dm0[:].broadcast_to((P, STW * N)),
                        st[:],
                        -1.0,
                        None,
                        Alu.mult,
                        Alu.max,
                        accum_out=minp[:, s : s + 1],
                    )
                    dm1 = dmyp.tile([P, 1], f16, tag="dmy")
                    nc.vector.tensor_scalar(
                        dm1[:].broadcast_to((P, STW * N)),
                        st[:],
                        1.0,
                        None,
                        Alu.mult,
                        Alu.max,
                        accum_out=maxp[:, s : s + 1],
                    )
                    sts.append(st)

                # ---------------- phase C: finalize scalars ----------------
                s2 = smallp.tile([P, 2], f32, tag="s2")
                sr = smallp.tile([P, 2], f32, tag="sr")
                u = smallp.tile([P, 1], f32, tag="u")
                r = smallp.tile([P, 1], f32, tag="r")
                c1 = smallp.tile([P, 1], f32, tag="c1")
                t0 = smallp.tile([P, 1], f32, tag="t0")
                c0 = smallp.tile([P, 1], f32, tag="c0")

                nc.vector.tensor_reduce(s2[:, 0:1], minp[:], AX.X, Alu.max)
                nc.vector.tensor_reduce(s2[:, 1:2], maxp[:], AX.X, Alu.max)
                nc.gpsimd.partition_all_reduce(
                    sr[:], s2[:], P, bass_isa.ReduceOp.max
                )
                nmn = sr[:, 0:1]  # -min, on every partition
                mx = sr[:, 1:2]  # max, on every partition
                # c1 = (A_HI-A_LO)/(mx-mn);  c0 = A_LO - mn*c1 = A_LO + nmn*c1
                nc.vector.tensor_tensor(u[:], mx, nmn, Alu.add)  # mx - mn
                nc.vector.reciprocal(r[:], u[:])
                nc.vector.tensor_scalar(c1[:], r[:], A_HI - A_LO, None, Alu.mult)
                nc.vector.tensor_tensor(t0[:], nmn, c1[:], Alu.mult)
                nc.vector.tensor_scalar(c0[:], t0[:], A_LO, None, Alu.add)

                # ---------------- phase D: affine + store ----------------
                for s in range(ST):
                    st = sts[s]
                    if (b * ST + s) % ST < AFF_ACT:
                        nc.scalar.activation(
                            st[:],
                            st[:],
                            AF.Identity,
                            bias=c0[:, 0:1],
                            scale=c1[:, 0:1],
                        )
                    else:
                        nc.vector.tensor_scalar(
                            st[:], st[:], c1[:, 0:1], c0[:, 0:1], Alu.mult, Alu.add
                        )
                    nc.sync.dma_start(
                        Od[b, s * STW * P : (s + 1) * STW * P, :].rearrange(
                            "(t p) n -> p t n", p=P
                        ),
                        st[:].rearrange("p (t n) -> p t n", n=N),
                    )

    nc.compile()
    return nc


def get_program():
    if "nc" not in _CACHE:
        _CACHE["nc"] = build_program()
    return _CACHE["nc"]


def run(inputs, trace=False):
    """Run on 8 cores; returns (out [H,B,N,N] f32, BassKernelResults)."""
    Q = np.ascontiguousarray(np.asarray(inputs["Q"], dtype=np.float32))
    K = np.ascontiguousarray(np.asarray(inputs["K"], dtype=np.float32))
    assert Q.shape == (H, B, N, D) and K.shape == (H, B, N, D)

    from concourse.bass_utils import run_bass_kernel_spmd

    nc = get_program()
    in_maps = [{"Q": Q[h], "K": K[h]} for h in range(H)]
    res = run_bass_kernel_spmd(nc, in_maps, core_ids=list(range(H)), trace=trace)
    out = np.stack(
        [np.asarray(res.results[h]["out"]) for h in range(H)], axis=0
    ).astype(np.float32)
    return out, res


def kernel(**inputs) -> np.ndarray:
    out, _ = run(inputs, trace=False)
    return out


if __name__ == "__main__":
    # quick smoke: build only
    nc = get_program()
    print("build ok:", nc)


# revision 7
# speedup vs baseline: 1.0146x; 1.0146x over previous
"""Trainium2 Bass kernel: per-(head,batch) euclidean compatibility matrix,
globally min/max-rescaled to [-9, 9].

reference (jax):
    q_sq = sum(Q*Q, -1)[..., :, None]
    k_sq = sum(K*K, -1)[..., None, :]
    cross = einsum("hbqd,hbgd->hbqg", Q, K)
    compat = sqrt(q_sq + k_sq - 2*cross)
    out = A_LO + (compat - min) * (A_HI - A_LO) / (max - min)   # min/max per (h,b)

Sharding: head h -> NeuronCore h (8 heads, 8 cores), fully independent.

Per-core program (B=4 slices of [N=2048, D=16]):
  phase A (all slices up front): load Q/K in natural layout [128, 16, 18]
    (fp32), compute row sum-of-squares; K gets (k_hi, k_lo) fp16-split
    sum-of-squares columns, Q gets const -0.5 columns and a separate
    per-partition q_sq tile (fp32, fed to ACT as bias).  PE-transposes
    build fp16 UT = [Q^T; -.5; -.5], VT = [K^T; k_hi; k_lo], so
      psum = UT[:,q]^T @ VT[:,g] = QK - 0.5*k_sq
      d2   = -2*psum + q_sq      (exact-ish; ACT pre-affine, scale=-2)
  phase B: per supertile (4 q-tiles, [128, 8192] fp16): 16 fp16 matmuls
    -> PSUM, ACT sqrt(scale*x+bias) PSUM->fp16 SBUF, 2 fused DVE
    tensor_scalar+accum reduces (min via negate / max partials).
  phase C: finalize min/max across supertiles + partitions (gpsimd
    all-reduce), c1 = 18/(max-min), c0 = -9 + (-min)*c1.
  phase D: per supertile: DVE madd (fp16, 4x mode), DMA to DRAM fp16.

Output is fp16 on device (halves HBM write traffic); host upcasts to
fp32.  rel-err budget 2e-2 >> fp16 quantization (~1e-3).
"""

import numpy as np

H, B, N, D = 8, 4, 2048, 16
A_LO, A_HI = -9.0, 9.0
P = 128
R = D + 2            # matmul contraction rows: 16 data + k_hi + k_lo
NT = N // P          # 16 q-tiles per slice
STW = 4              # q-tiles per supertile
ST = NT // STW       # 4 supertiles per slice

# ---- tuning knobs ----
SQ_BUFS = 8          # SBUF supertile bufs of [128, 8192] fp16
ACT_COPIES = 8       # of the 8 phase-A PSUM->SBUF copies routed to ACT (rest DVE)
AFF_ACT = 0          # of ST*B affines routed to ACT (rest DVE)
MM_N = 512           # matmul moving free dim (1 PSUM bank; 1024 fails ISA check)

_CACHE = {}


def build_program():
    import concourse.bacc as bacc
    import concourse.bass as bass
    import concourse.mybir as mybir
    from concourse import tile, masks
    from concourse import bass_isa

    f32 = mybir.dt.float32
    f16 = mybir.dt.float16
    Alu = mybir.AluOpType
    AF = mybir.ActivationFunctionType
    AX = mybir.AxisListType

    nc = bacc.Bacc()
    Qd = nc.declare_dram_parameter("Q", [B, N, D], f32, isOutput=False)
    Kd = nc.declare_dram_parameter("K", [B, N, D], f32, isOutput=False)
    Od = nc.declare_dram_parameter("out", [B, N, N], f16, isOutput=True)

    with tile.TileContext(nc) as tc:
        with (
            tc.tile_pool(name="const", bufs=1) as constp,
            tc.tile_pool(name="ld", bufs=2) as ldp,
            tc.tile_pool(name="sqt", bufs=2) as sqtp,
            tc.tile_pool(name="qsq", bufs=4) as qsqp,
            tc.tile_pool(name="ksq", bufs=2) as ksqp,
            tc.tile_pool(name="uv", bufs=8) as uvp,
            tc.tile_pool(name="sq", bufs=SQ_BUFS) as sqp,
            tc.tile_pool(name="dmy", bufs=2) as dmyp,
            tc.tile_pool(name="small", bufs=2) as smallp,
            tc.tile_pool(name="ps", bufs=2, space=bass.MemorySpace.PSUM) as psp,
        ):
            ident = constp.tile([P, P], f32)
            masks.make_identity(nc, ident[:])

            # ---------------- phase A: build UT / VT / qsq for all slices ----
            UTs, VTs, qsqs = [], [], []
            copy_idx = 0
            for b in range(B):
                for (src, is_k) in ((Qd, False), (Kd, True)):
                    ld = ldp.tile([P, NT, R], f32, tag="ld")
                    if not is_k:
                        # const -0.5 in cols D, D+1 (survives the data DMA)
                        nc.gpsimd.memset(ld[:], -0.5)
                    nc.sync.dma_start(
                        ld[:, :, 0:D], src[b].rearrange("(t p) d -> p t d", p=P)
                    )
                    sqt = sqtp.tile([P, NT, D], f32, tag="sqt")
                    nc.vector.tensor_tensor(
                        sqt[:], ld[:, :, 0:D], ld[:, :, 0:D], Alu.mult
                    )
                    if is_k:
                        ksq = ksqp.tile([P, NT], f32, tag="ksq")
                        nc.vector.tensor_reduce(ksq[:], sqt[:], AX.X, Alu.add)
                        # fp16 hi/lo split of k_sq -> ld cols D, D+1
                        khil = ksqp.tile([P, NT], f16, tag="khil")
                        nc.vector.tensor_copy(khil[:], ksq[:])
                        nc.vector.tensor_copy(ld[:, :, D], khil[:])
                        nc.vector.tensor_tensor(
                            ld[:, :, D + 1], ksq[:], ld[:, :, D], Alu.subtract
                        )
                    else:
                        qsq = qsqp.tile([P, NT], f32, tag="qsq")
                        nc.vector.tensor_reduce(qsq[:], sqt[:], AX.X, Alu.add)
                        qsqs.append(qsq)
                    # transpose [128, 18] chunks -> [18, 2048] fp16 in SBUF
                    tp = psp.tile([R, N], f32, tag="ps")
                    for t in range(NT):
                        nc.tensor.transpose(
                            tp[:, t * P : (t + 1) * P], ld[:, t, :], ident[:]
                        )
                    TT = uvp.tile([R, N], f16, tag="uv")
                    if copy_idx < ACT_COPIES:
                        nc.scalar.copy(TT[:], tp[:])
                    else:
                        nc.vector.tensor_copy(TT[:], tp[:])
                    copy_idx += 1
                    (VTs if is_k else UTs).append(TT)

            # ---------------- phases B-D per slice ----------------
            for b in range(B):
                UT, VT, qsq = UTs[b], VTs[b], qsqs[b]
                minp = smallp.tile([P, ST], f32, tag="minp")
                maxp = smallp.tile([P, ST], f32, tag="maxp")
                sts = []
                for s in range(ST):
                    st = sqp.tile([P, STW * N], f16, tag="sq")
                    for t in range(STW):
                        i = s * STW + t
                        d2 = psp.tile([P, N], f32, tag="ps")
                        lhs = UT[:, i * P : (i + 1) * P]
                        for j in range(N // MM_N):
                            nc.tensor.matmul(
                                d2[:, j * MM_N : (j + 1) * MM_N],
                                lhs,
                                VT[:, j * MM_N : (j + 1) * MM_N],
                                start=True,
                                stop=True,
                            )
                        # sq = sqrt(-2*psum + q_sq) = euclidean distance
                        nc.scalar.activation(
                            st[:, t * N : (t + 1) * N],
                            d2[:],
                            AF.Sqrt,
                            bias=qsq[:, i : i + 1],
                            scale=-2.0,
                        )
                    # fused reduces: minp holds NEGATED minima (max of -sq).
                    # A real step-1 fp16 output (junk tile) keeps the DVE in
                    # 4x perf mode; a stride-0 dummy output forces 1x.
                    junk = dmyp.tile([P, STW * N], f16, tag="junk", bufs=1)
                    nc.vector.tensor_scalar(
                        junk[:],
                        st[:],
                        -1.0,
                        None,
                        Alu.mult,
                        Alu.max,
                        accum_out=minp[:, s : s + 1],
                    )
                    nc.vector.tensor_scalar(
                        junk[:],
                        st[:],
                        1.0,
                        None,
                        Alu.mult,
                        Alu.max,
                        accum_out=maxp[:, s : s + 1],
                    )
                    sts.append(st)

                # ---------------- phase C: finalize scalars ----------------
                s2 = smallp.tile([P, 2], f32, tag="s2")
                sr = smallp.tile([P, 2], f32, tag="sr")
                u = smallp.tile([P, 1], f32, tag="u")
                r = smallp.tile([P, 1], f32, tag="r")
                c1 = smallp.tile([P, 1], f32, tag="c1")
                t0 = smallp.tile([P, 1], f32, tag="t0")
                c0 = smallp.tile([P, 1], f32, tag="c0")

                nc.vector.tensor_reduce(s2[:, 0:1], minp[:], AX.X, Alu.max)
                nc.vector.tensor_reduce(s2[:, 1:2], maxp[:], AX.X, Alu.max)
                nc.gpsimd.partition_all_reduce(
                    sr[:], s2[:], P, bass_isa.ReduceOp.max
                )
                nmn = sr[:, 0:1]  # -min, on every partition
                mx = sr[:, 1:2]  # max, on every partition
                # c1 = (A_HI-A_LO)/(mx-mn);  c0 = A_LO - mn*c1 = A_LO + nmn*c1
                nc.vector.tensor_tensor(u[:], mx, nmn, Alu.add)  # mx - mn
                nc.vector.reciprocal(r[:], u[:])
                nc.vector.tensor_scalar(c1[:], r[:], A_HI - A_LO, None, Alu.mult)
                nc.vector.tensor_tensor(t0[:], nmn, c1[:], Alu.mult)
                nc.vector.tensor_scalar(c0[:], t0[:], A_LO, None, Alu.add)

                # ---------------- phase D: affine + store ----------------
                for s in range(ST):
                    st = sts[s]
                    if (b * ST + s) % ST < AFF_ACT:
                        nc.scalar.activation(
                            st[:],
                            st[:],
                            AF.Identity,
                            bias=c0[:, 0:1],
                            scale=c1[:, 0:1],
                        )
                    else:
                        nc.vector.tensor_scalar(
                            st[:], st[:], c1[:, 0:1], c0[:, 0:1], Alu.mult, Alu.add
                        )
                    nc.sync.dma_start(
                        Od[b, s * STW * P : (s + 1) * STW * P, :].rearrange(
                            "(t p) n -> p t n", p=P
                        ),
                        st[:].rearrange("p (t n) -> p t n", n=N),
                    )

    nc.compile()
    return nc


def get_program():
    if "nc" not in _CACHE:
        _CACHE["nc"] = build_program()
    return _CACHE["nc"]


def run(inputs, trace=False):
    """Run on 8 cores; returns (out [H,B,N,N] f32, BassKernelResults)."""
    Q = np.ascontiguousarray(np.asarray(inputs["Q"], dtype=np.float32))
    K = np.ascontiguousarray(np.asarray(inputs["K"], dtype=np.float32))
    assert Q.shape == (H, B, N, D) and K.shape == (H, B, N, D)

    from concourse.bass_utils import run_bass_kernel_spmd

    nc = get_program()
    in_maps = [{"Q": Q[h], "K": K[h]} for h in range(H)]
    res = run_bass_kernel_spmd(nc, in_maps, core_ids=list(range(H)), trace=trace)
    out = np.stack(
        [np.asarray(res.results[h]["out"]) for h in range(H)], axis=0
    ).astype(np.float32)
    return out, res


def kernel(**inputs) -> np.ndarray:
    out, _ = run(inputs, trace=False)
    return out


if __name__ == "__main__":
    # quick smoke: build only
    nc = get_program()
    print("build ok:", nc)


# revision 11
# speedup vs baseline: 1.5058x; 1.4841x over previous
"""Trainium2 Bass kernel: per-(head,batch) euclidean compatibility matrix,
globally min/max-rescaled to [-9, 9].

reference (jax):
    q_sq = sum(Q*Q, -1)[..., :, None]
    k_sq = sum(K*K, -1)[..., None, :]
    cross = einsum("hbqd,hbgd->hbqg", Q, K)
    compat = sqrt(q_sq + k_sq - 2*cross)
    out = A_LO + (compat - min) * (A_HI - A_LO) / (max - min)   # min/max per (h,b)

Sharding: head h -> NeuronCore h (8 heads, 8 cores), fully independent.

Per-core program (B=4 slices of [N=2048, D=16]):
  phase A (all slices up front): load Q/K in natural layout [128, 16, 18]
    (fp32), compute row sum-of-squares; K gets (k_hi, k_lo) fp16-split
    sum-of-squares columns, Q gets const -0.5 columns and a separate
    per-partition q_sq tile (fp32, fed to ACT as bias).  PE-transposes
    build fp16 UT = [Q^T; -.5; -.5], VT = [K^T; k_hi; k_lo], so
      psum = UT[:,q]^T @ VT[:,g] = QK - 0.5*k_sq
      d2   = -2*psum + q_sq      (exact-ish; ACT pre-affine, scale=-2)
  phase B: per supertile (4 q-tiles, [128, 8192] fp16): 16 fp16 matmuls
    -> PSUM, ACT sqrt(scale*x+bias) PSUM->fp16 SBUF, 2 fused DVE
    tensor_scalar+accum reduces (min via negate / max partials).
  phase C: finalize min/max across supertiles + partitions (gpsimd
    all-reduce), c1 = 18/(max-min), c0 = -9 + (-min)*c1.
  phase D: per supertile: DVE madd (fp16, 4x mode), DMA to DRAM fp16.

Output is fp16 on device (halves HBM write traffic); host upcasts to
fp32.  rel-err budget 2e-2 >> fp16 quantization (~1e-3).
"""

import numpy as np

H, B, N, D = 8, 4, 2048, 16
A_LO, A_HI = -9.0, 9.0
P = 128
R = D + 2            # matmul contraction rows: 16 data + k_hi + k_lo
NT = N // P          # 16 q-tiles per slice
STW = 4              # q-tiles per supertile
ST = NT // STW       # 4 supertiles per slice

# ---- tuning knobs ----
SQ_BUFS = 8          # SBUF supertile bufs of [128, 8192] fp16
ACT_COPIES = 8       # of the 8 phase-A PSUM->SBUF copies routed to ACT (rest DVE)
AFF_EVERY = 2        # every Nth supertile affine on ACT (0 = all on DVE)
MM_N = 512           # matmul moving free dim (1 PSUM bank; 1024 fails ISA check)

_CACHE = {}


def build_program():
    import concourse.bacc as bacc
    import concourse.bass as bass
    import concourse.mybir as mybir
    from concourse import tile, masks
    from concourse import bass_isa

    f32 = mybir.dt.float32
    f16 = mybir.dt.float16
    Alu = mybir.AluOpType
    AF = mybir.ActivationFunctionType
    AX = mybir.AxisListType

    nc = bacc.Bacc()
    Qd = nc.declare_dram_parameter("Q", [B, N, D], f32, isOutput=False)
    Kd = nc.declare_dram_parameter("K", [B, N, D], f32, isOutput=False)
    Od = nc.declare_dram_parameter("out", [B, N, N], f16, isOutput=True)

    with tile.TileContext(nc) as tc:
        with (
            tc.tile_pool(name="const", bufs=1) as constp,
            tc.tile_pool(name="ld", bufs=2) as ldp,
            tc.tile_pool(name="sqt", bufs=2) as sqtp,
            tc.tile_pool(name="qsq", bufs=4) as qsqp,
            tc.tile_pool(name="ksq", bufs=2) as ksqp,
            tc.tile_pool(name="uv", bufs=8) as uvp,
            tc.tile_pool(name="sq", bufs=SQ_BUFS) as sqp,
            tc.tile_pool(name="scr", bufs=1) as scrp,
            tc.tile_pool(name="small", bufs=2) as smallp,
            tc.tile_pool(name="ps", bufs=2, space=bass.MemorySpace.PSUM) as psp,
        ):
            ident = constp.tile([P, P], f32)
            masks.make_identity(nc, ident[:])

            # ---------------- phase A: build UT / VT / qsq for all slices ----
            UTs, VTs, qsqs = [], [], []
            copy_idx = 0
            for b in range(B):
                for (src, is_k) in ((Qd, False), (Kd, True)):
                    ld = ldp.tile([P, NT, R], f32, tag="ld")
                    if not is_k:
                        # const -0.5 in cols D, D+1 (survives the data DMA)
                        nc.gpsimd.memset(ld[:], -0.5)
                    nc.sync.dma_start(
                        ld[:, :, 0:D], src[b].rearrange("(t p) d -> p t d", p=P)
                    )
                    sqt = sqtp.tile([P, NT, D], f32, tag="sqt")
                    nc.vector.tensor_tensor(
                        sqt[:], ld[:, :, 0:D], ld[:, :, 0:D], Alu.mult
                    )
                    if is_k:
                        ksq = ksqp.tile([P, NT], f32, tag="ksq")
                        nc.vector.tensor_reduce(ksq[:], sqt[:], AX.X, Alu.add)
                        # fp16 hi/lo split of k_sq -> ld cols D, D+1
                        khil = ksqp.tile([P, NT], f16, tag="khil")
                        nc.vector.tensor_copy(khil[:], ksq[:])
                        nc.vector.tensor_copy(ld[:, :, D], khil[:])
                        nc.vector.tensor_tensor(
                            ld[:, :, D + 1], ksq[:], ld[:, :, D], Alu.subtract
                        )
                    else:
                        qsq = qsqp.tile([P, NT], f32, tag="qsq")
                        nc.vector.tensor_reduce(qsq[:], sqt[:], AX.X, Alu.add)
                        qsqs.append(qsq)
                    # transpose [128, 18] chunks -> [18, 2048] fp16 in SBUF
                    tp = psp.tile([R, N], f32, tag="ps")
                    for t in range(NT):
                        nc.tensor.transpose(
                            tp[:, t * P : (t + 1) * P], ld[:, t, :], ident[:]
                        )
                    TT = uvp.tile([R, N], f16, tag="uv")
                    if copy_idx < ACT_COPIES:
                        nc.scalar.copy(TT[:], tp[:])
                    else:
                        nc.vector.tensor_copy(TT[:], tp[:])
                    copy_idx += 1
                    (VTs if is_k else UTs).append(TT)

            # ---------------- phases B-D per slice ----------------
            for b in range(B):
                UT, VT, qsq = UTs[b], VTs[b], qsqs[b]
                minp = smallp.tile([P, ST], f32, tag="minp")
                maxp = smallp.tile([P, ST], f32, tag="maxp")
                sts = []
                for s in range(ST):
                    st = sqp.tile([P, STW * N], f16, tag="sq")
                    for t in range(STW):
                        i = s * STW + t
                        d2 = psp.tile([P, N], f32, tag="ps")
                        lhs = UT[:, i * P : (i + 1) * P]
                        for j in range(N // MM_N):
                            nc.tensor.matmul(
                                d2[:, j * MM_N : (j + 1) * MM_N],
                                lhs,
                                VT[:, j * MM_N : (j + 1) * MM_N],
                                start=True,
                                stop=True,
                            )
                        # sq = sqrt(-2*psum + q_sq) = euclidean distance
                        nc.scalar.activation(
                            st[:, t * N : (t + 1) * N],
                            d2[:],
                            AF.Sqrt,
                            bias=qsq[:, i : i + 1],
                            scale=-2.0,
                        )
                    # min/max via fp16 TT fold trees (2x mode; accum ops are
                    # stuck at 1x for 16-bit so fold first, accum on 512).
                    # minp holds NEGATED minima (max of -x at the accum).
                    for (alu, sgn, partial) in (
                        (Alu.max, 1.0, maxp),
                        (Alu.min, -1.0, minp),
                    ):
                        t1 = scrp.tile([P, 4096], f16, tag="t1")
                        nc.vector.tensor_tensor(
                            t1[:], st[:, 0:4096], st[:, 4096:8192], alu
                        )
                        t2 = scrp.tile([P, 2048], f16, tag="t2")
                        nc.vector.tensor_tensor(
                            t2[:], t1[:, 0:2048], t1[:, 2048:4096], alu
                        )
                        t3 = scrp.tile([P, 1024], f16, tag="t3")
                        nc.vector.tensor_tensor(
                            t3[:], t2[:, 0:1024], t2[:, 1024:2048], alu
                        )
                        t4 = scrp.tile([P, 512], f16, tag="t4")
                        nc.vector.tensor_tensor(
                            t4[:], t3[:, 0:512], t3[:, 512:1024], alu
                        )
                        t5 = scrp.tile([P, 512], f16, tag="t5")
                        nc.vector.tensor_scalar(
                            t5[:],
                            t4[:],
                            sgn,
                            None,
                            Alu.mult,
                            Alu.max,
                            accum_out=partial[:, s : s + 1],
                        )
                    sts.append(st)

                # ---------------- phase C: finalize scalars ----------------
                s2 = smallp.tile([P, 2], f32, tag="s2")
                sr = smallp.tile([P, 2], f32, tag="sr")
                u = smallp.tile([P, 1], f32, tag="u")
                r = smallp.tile([P, 1], f32, tag="r")
                c1 = smallp.tile([P, 1], f32, tag="c1")
                t0 = smallp.tile([P, 1], f32, tag="t0")
                c0 = smallp.tile([P, 1], f32, tag="c0")

                nc.vector.tensor_reduce(s2[:, 0:1], minp[:], AX.X, Alu.max)
                nc.vector.tensor_reduce(s2[:, 1:2], maxp[:], AX.X, Alu.max)
                nc.gpsimd.partition_all_reduce(
                    sr[:], s2[:], P, bass_isa.ReduceOp.max
                )
                nmn = sr[:, 0:1]  # -min, on every partition
                mx = sr[:, 1:2]  # max, on every partition
                # c1 = (A_HI-A_LO)/(mx-mn);  c0 = A_LO - mn*c1 = A_LO + nmn*c1
                nc.vector.tensor_tensor(u[:], mx, nmn, Alu.add)  # mx - mn
                nc.vector.reciprocal(r[:], u[:])
                nc.vector.tensor_scalar(c1[:], r[:], A_HI - A_LO, None, Alu.mult)
                nc.vector.tensor_tensor(t0[:], nmn, c1[:], Alu.mult)
                nc.vector.tensor_scalar(c0[:], t0[:], A_LO, None, Alu.add)

                # ---------------- phase D: affine + store ----------------
                for s in range(ST):
                    st = sts[s]
                    if AFF_EVERY and (b * ST + s) % AFF_EVERY == 0:
                        nc.scalar.activation(
                            st[:],
                            st[:],
                            AF.Identity,
                            bias=c0[:, 0:1],
                            scale=c1[:, 0:1],
                        )
                    else:
                        nc.vector.tensor_scalar(
                            st[:], st[:], c1[:, 0:1], c0[:, 0:1], Alu.mult, Alu.add
                        )
                    nc.sync.dma_start(
                        Od[b, s * STW * P : (s + 1) * STW * P, :].rearrange(
                            "(t p) n -> p t n", p=P
                        ),
                        st[:].rearrange("p (t n) -> p t n", n=N),
                    )

    nc.compile()
    return nc


def get_program():
    if "nc" not in _CACHE:
        _CACHE["nc"] = build_program()
    return _CACHE["nc"]


def run(inputs, trace=False):
    """Run on 8 cores; returns (out [H,B,N,N] f32, BassKernelResults)."""
    Q = np.ascontiguousarray(np.asarray(inputs["Q"], dtype=np.float32))
    K = np.ascontiguousarray(np.asarray(inputs["K"], dtype=np.float32))
    assert Q.shape == (H, B, N, D) and K.shape == (H, B, N, D)

    from concourse.bass_utils import run_bass_kernel_spmd

    nc = get_program()
    in_maps = [{"Q": Q[h], "K": K[h]} for h in range(H)]
    res = run_bass_kernel_spmd(nc, in_maps, core_ids=list(range(H)), trace=trace)
    out = np.stack(
        [np.asarray(res.results[h]["out"]) for h in range(H)], axis=0
    ).astype(np.float32)
    return out, res


def kernel(**inputs) -> np.ndarray:
    out, _ = run(inputs, trace=False)
    return out


if __name__ == "__main__":
    # quick smoke: build only
    nc = get_program()
    print("build ok:", nc)


# revision 17
# speedup vs baseline: 1.5499x; 1.0293x over previous
"""Trainium2 Bass kernel: per-(head,batch) euclidean compatibility matrix,
globally min/max-rescaled to [-9, 9].

reference (jax):
    q_sq = sum(Q*Q, -1)[..., :, None]
    k_sq = sum(K*K, -1)[..., None, :]
    cross = einsum("hbqd,hbgd->hbqg", Q, K)
    compat = sqrt(q_sq + k_sq - 2*cross)
    out = A_LO + (compat - min) * (A_HI - A_LO) / (max - min)   # min/max per (h,b)

Sharding: head h -> NeuronCore h (8 heads, 8 cores), fully independent.

Per-core program (B=4 slices of [N=2048, D=16]):
  phase A (all slices up front): load Q/K in natural layout [128, 16, 18]
    (fp32), compute row sum-of-squares; K gets (k_hi, k_lo) fp16-split
    sum-of-squares columns, Q gets const -0.5 columns and a separate
    per-partition q_sq tile (fp32, fed to ACT as bias).  PE-transposes
    build fp16 UT = [Q^T; -.5; -.5], VT = [K^T; k_hi; k_lo], so
      psum = UT[:,q]^T @ VT[:,g] = QK - 0.5*k_sq
      d2   = -2*psum + q_sq      (exact-ish; ACT pre-affine, scale=-2)
  phase B: per supertile (4 q-tiles, [128, 8192] fp16): 16 fp16 matmuls
    -> PSUM, ACT sqrt(scale*x+bias) PSUM->fp16 SBUF, 2 fused DVE
    tensor_scalar+accum reduces (min via negate / max partials).
  phase C: finalize min/max across supertiles + partitions (gpsimd
    all-reduce), c1 = 18/(max-min), c0 = -9 + (-min)*c1.
  phase D: per supertile: DVE madd (fp16, 4x mode), DMA to DRAM fp16.

Output is fp16 on device (halves HBM write traffic); host upcasts to
fp32.  rel-err budget 2e-2 >> fp16 quantization (~1e-3).
"""

import numpy as np

H, B, N, D = 8, 4, 2048, 16
A_LO, A_HI = -9.0, 9.0
P = 128
R = D + 2            # matmul contraction rows: 16 data + k_hi + k_lo
NT = N // P          # 16 q-tiles per slice
STW = 4              # q-tiles per supertile
ST = NT // STW       # 4 supertiles per slice

# ---- tuning knobs ----
SQ_BUFS = 8          # SBUF supertile bufs of [128, 8192] fp16
ACT_COPIES = 8       # of the 8 phase-A PSUM->SBUF copies routed to ACT (rest DVE)
AFF_EVERY = 2        # every Nth supertile affine on ACT (0 = all on DVE)
MM_N = 512           # matmul moving free dim (1 PSUM bank; 1024 fails ISA check)

_CACHE = {}


def build_program():
    import concourse.bacc as bacc
    import concourse.bass as bass
    import concourse.mybir as mybir
    from concourse import tile, masks
    from concourse import bass_isa

    f32 = mybir.dt.float32
    f16 = mybir.dt.float16
    Alu = mybir.AluOpType
    AF = mybir.ActivationFunctionType
    AX = mybir.AxisListType

    nc = bacc.Bacc()
    Qd = nc.declare_dram_parameter("Q", [B, N, D], f32, isOutput=False)
    Kd = nc.declare_dram_parameter("K", [B, N, D], f32, isOutput=False)
    Od = nc.declare_dram_parameter("out", [B, N, N], f16, isOutput=True)

    with tile.TileContext(nc) as tc:
        with (
            tc.tile_pool(name="const", bufs=1) as constp,
            tc.tile_pool(name="ld", bufs=8) as ldp,
            tc.tile_pool(name="sqt", bufs=2) as sqtp,
            tc.tile_pool(name="qsq", bufs=4) as qsqp,
            tc.tile_pool(name="ksq", bufs=2) as ksqp,
            tc.tile_pool(name="uv", bufs=8) as uvp,
            tc.tile_pool(name="sq", bufs=SQ_BUFS) as sqp,
            tc.tile_pool(name="scr", bufs=1) as scrp,
            tc.tile_pool(name="small", bufs=2) as smallp,
            tc.tile_pool(name="ps", bufs=2, space=bass.MemorySpace.PSUM) as psp,
        ):
            ident = constp.tile([P, P], f32)
            masks.make_identity(nc, ident[:])

            # ---------------- phase A: build UT / VT / qsq for all slices ----
            # prefetch: all 8 input DMAs in flight up front
            lds = []
            for b in range(B):
                for (src, is_k) in ((Qd, False), (Kd, True)):
                    ld = ldp.tile([P, NT, R], f32, tag="ld")
                    if not is_k:
                        # const -0.5 in cols D, D+1 (survives the data DMA)
                        nc.gpsimd.memset(ld[:], -0.5)
                    nc.sync.dma_start(
                        ld[:, :, 0:D], src[b].rearrange("(t p) d -> p t d", p=P)
                    )
                    lds.append(ld)
            UTs, VTs, qsqs = [], [], []
            copy_idx = 0
            for b in range(B):
                for (src, is_k) in ((Qd, False), (Kd, True)):
                    ld = lds[b * 2 + (1 if is_k else 0)]
                    sqt = sqtp.tile([P, NT, D], f32, tag="sqt")
                    nc.vector.tensor_tensor(
                        sqt[:], ld[:, :, 0:D], ld[:, :, 0:D], Alu.mult
                    )
                    if is_k:
                        ksq = ksqp.tile([P, NT], f32, tag="ksq")
                        nc.vector.tensor_reduce(ksq[:], sqt[:], AX.X, Alu.add)
                        # fp16 hi/lo split of k_sq -> ld cols D, D+1
                        khil = ksqp.tile([P, NT], f16, tag="khil")
                        nc.vector.tensor_copy(khil[:], ksq[:])
                        nc.vector.tensor_copy(ld[:, :, D], khil[:])
                        nc.vector.tensor_tensor(
                            ld[:, :, D + 1], ksq[:], ld[:, :, D], Alu.subtract
                        )
                    else:
                        qsq = qsqp.tile([P, NT], f32, tag="qsq")
                        nc.vector.tensor_reduce(qsq[:], sqt[:], AX.X, Alu.add)
                        qsqs.append(qsq)
                    # transpose [128, 18] chunks -> [18, 2048] fp16 in SBUF
                    tp = psp.tile([R, N], f32, tag="ps")
                    for t in range(NT):
                        nc.tensor.transpose(
                            tp[:, t * P : (t + 1) * P], ld[:, t, :], ident[:]
                        )
                    TT = uvp.tile([R, N], f16, tag="uv")
                    if copy_idx < ACT_COPIES:
                        nc.scalar.copy(TT[:], tp[:])
                    else:
                        nc.vector.tensor_copy(TT[:], tp[:])
                    copy_idx += 1
                    (VTs if is_k else UTs).append(TT)

            # ---------------- phases B-D per slice ----------------
            def emit_phase_d(b, sts, c0, c1):
                # affine + store, deferred one slice so the in-order ACT/DVE
                # queues never park slice b's affines ahead of slice b+1's
                # compute while finalize is still pending.
                for s in range(ST):
                    st = sts[s]
                    if AFF_EVERY and (b * ST + s) % AFF_EVERY == 0:
                        nc.scalar.activation(
                            st[:],
                            st[:],
                            AF.Identity,
                            bias=c0[:, 0:1],
                            scale=c1[:, 0:1],
                        )
                    else:
                        nc.vector.tensor_scalar(
                            st[:], st[:], c1[:, 0:1], c0[:, 0:1], Alu.mult, Alu.add
                        )
                    nc.sync.dma_start(
                        Od[b, s * STW * P : (s + 1) * STW * P, :].rearrange(
                            "(t p) n -> p t n", p=P
                        ),
                        st[:].rearrange("p (t n) -> p t n", n=N),
                    )

            pend = None
            for b in range(B):
                UT, VT, qsq = UTs[b], VTs[b], qsqs[b]
                minp = smallp.tile([P, ST], f32, tag="minp")
                maxp = smallp.tile([P, ST], f32, tag="maxp")
                sts = []
                for s in range(ST):
                    st = sqp.tile([P, STW * N], f16, tag="sq")
                    for t in range(STW):
                        i = s * STW + t
                        d2 = psp.tile([P, N], f32, tag="ps")
                        lhs = UT[:, i * P : (i + 1) * P]
                        for j in range(N // MM_N):
                            nc.tensor.matmul(
                                d2[:, j * MM_N : (j + 1) * MM_N],
                                lhs,
                                VT[:, j * MM_N : (j + 1) * MM_N],
                                start=True,
                                stop=True,
                            )
                        # sq = sqrt(-2*psum + q_sq) = euclidean distance
                        nc.scalar.activation(
                            st[:, t * N : (t + 1) * N],
                            d2[:],
                            AF.Sqrt,
                            bias=qsq[:, i : i + 1],
                            scale=-2.0,
                        )
                    # min/max via fp16 TT fold trees (2x mode; accum ops are
                    # stuck at 1x for 16-bit so fold first, accum on 512).
                    # minp holds NEGATED minima (max of -x at the accum).
                    for (alu, sgn, partial) in (
                        (Alu.max, 1.0, maxp),
                        (Alu.min, -1.0, minp),
                    ):
                        t1 = scrp.tile([P, 4096], f16, tag="t1")
                        nc.vector.tensor_tensor(
                            t1[:], st[:, 0:4096], st[:, 4096:8192], alu
                        )
                        t2 = scrp.tile([P, 2048], f16, tag="t2")
                        nc.vector.tensor_tensor(
                            t2[:], t1[:, 0:2048], t1[:, 2048:4096], alu
                        )
                        t3 = scrp.tile([P, 1024], f16, tag="t3")
                        nc.vector.tensor_tensor(
                            t3[:], t2[:, 0:1024], t2[:, 1024:2048], alu
                        )
                        t4 = scrp.tile([P, 512], f16, tag="t4")
                        nc.vector.tensor_tensor(
                            t4[:], t3[:, 0:512], t3[:, 512:1024], alu
                        )
                        t5 = scrp.tile([P, 512], f16, tag="t5")
                        nc.vector.tensor_scalar(
                            t5[:],
                            t4[:],
                            sgn,
                            None,
                            Alu.mult,
                            Alu.max,
                            accum_out=partial[:, s : s + 1],
                        )
                    sts.append(st)

                # ---------------- phase C: finalize scalars ----------------
                s2 = smallp.tile([P, 2], f32, tag="s2")
                sr = smallp.tile([P, 2], f32, tag="sr")
                u = smallp.tile([P, 1], f32, tag="u")
                r = smallp.tile([P, 1], f32, tag="r")
                c1 = smallp.tile([P, 1], f32, tag="c1")
                t0 = smallp.tile([P, 1], f32, tag="t0")
                c0 = smallp.tile([P, 1], f32, tag="c0")

                nc.vector.tensor_reduce(s2[:, 0:1], minp[:], AX.X, Alu.max)
                nc.vector.tensor_reduce(s2[:, 1:2], maxp[:], AX.X, Alu.max)
                nc.gpsimd.partition_all_reduce(
                    sr[:], s2[:], P, bass_isa.ReduceOp.max
                )
                # previous slice's affines execute on DVE/ACT while gpsimd
                # runs the partition reduce for this slice
                if pend is not None:
                    emit_phase_d(*pend)
                    pend = None
                nmn = sr[:, 0:1]  # -min, on every partition
                mx = sr[:, 1:2]  # max, on every partition
                # c1 = (A_HI-A_LO)/(mx-mn);  c0 = A_LO - mn*c1 = A_LO + nmn*c1
                nc.vector.tensor_tensor(u[:], mx, nmn, Alu.add)  # mx - mn
                nc.vector.reciprocal(r[:], u[:])
                nc.vector.tensor_scalar(c1[:], r[:], A_HI - A_LO, None, Alu.mult)
                nc.vector.tensor_tensor(t0[:], nmn, c1[:], Alu.mult)
                nc.vector.tensor_scalar(c0[:], t0[:], A_LO, None, Alu.add)

                # ---------------- phase D: deferred one slice ----------------
                pend = (b, sts, c0, c1)
            emit_phase_d(*pend)

    nc.compile()
    return nc


def get_program():
    if "nc" not in _CACHE:
        _CACHE["nc"] = build_program()
    return _CACHE["nc"]


def run(inputs, trace=False):
    """Run on 8 cores; returns (out [H,B,N,N] f32, BassKernelResults)."""
    Q = np.ascontiguousarray(np.asarray(inputs["Q"], dtype=np.float32))
    K = np.ascontiguousarray(np.asarray(inputs["K"], dtype=np.float32))
    assert Q.shape == (H, B, N, D) and K.shape == (H, B, N, D)

    from concourse.bass_utils import run_bass_kernel_spmd

    nc = get_program()
    in_maps = [{"Q": Q[h], "K": K[h]} for h in range(H)]
    res = run_bass_kernel_spmd(nc, in_maps, core_ids=list(range(H)), trace=trace)
    out = np.stack(
        [np.asarray(res.results[h]["out"]) for h in range(H)], axis=0
    ).astype(np.float32)
    return out, res


def kernel(**inputs) -> np.ndarray:
    out, _ = run(inputs, trace=False)
    return out


if __name__ == "__main__":
    # quick smoke: build only
    nc = get_program()
    print("build ok:", nc)
